# revision 3
# baseline (speedup 1.0000x reference)
"""Multi-head attention (B=1, S=4096, D=768, H=12) on 8 trn2 NeuronCores — v3.

Sharding: 2D over (query rows x head halves): core c = (qb, hh) with
qb = c // 2 (1024 q rows), hh = c % 2 (6 heads = 384 model dims).

v3 over v2:
  - DMA order: weights/q/k/v stream first, the 8.4MB mask last, so the
    K/Q projections are not starved at t=0.
  - exp table set preloaded during the projection phase (dummy ACT op).
  - attention processes one head at a time: PSUM = 1 ctx bank + a
    4-ktile and a 2-ktile score slot (alternating, so the PE refills one
    while ACT drains the other) = 7 banks, leaving 1 bank for the V
    projection, whose chunks are emitted interleaved with the attention
    groups and fill PE gaps during the ACT-bound attention phase.
  - softmax: ACT reads scores straight from PSUM (exp fused with
    eviction, 1/sqrt(64) in the activation scale), DVE applies the {0,1}
    mask as one fp16 multiply per group, PV accumulates probs@V_aug
    (65th V column of ones -> denominator in ctx row 64).
  - O projection for query-half 0 is emitted right after its heads
    finish, overlapping query-half 1's attention.

Host sums the two hh partial outputs per q block and adds bo.
"""

import numpy as np

import concourse.bass as bass
import concourse.mybir as mybir
import concourse.tile as tile
from concourse import bacc, bass_utils

B, S, D, H = 1, 4096, 768, 12
DK = D // H  # 64
NCORES = 8
NQB, NHH = 4, 2
SQ = S // NQB  # 1024 query rows per core
HL = H // NHH  # 6 local heads
DL = D // NHH  # 384 local model dims
DTL = DL // 128  # 3 local dt tiles
DT = D // 128  # 6 contraction tiles
KT_TILES = S // 128  # 32 k tiles
NCH = S // 512  # 8 column chunks for full-seq projections

F16 = mybir.dt.float16
F32 = mybir.dt.float32

_CACHE = {}

# per-head k-tile groups: 4-ktile A slot / 2-ktile B slot alternating
GROUPS = []
_kt = 0
while _kt < KT_TILES:
    n = min(4 if len(GROUPS) % 2 == 0 else 3, KT_TILES - _kt)
    GROUPS.append(tuple(range(_kt, _kt + n)))
    _kt += n


def build_kernel(timing=False):
    nc = bacc.Bacc("TRN2", target_bir_lowering=False, debug=False, num_devices=NCORES)

    kw = {} if timing else {"kind": "ExternalInput"}
    qT = nc.dram_tensor("qT", [128, DT, SQ], F16, **kw)
    kT = nc.dram_tensor("kT", [128, DT, S], F16, **kw)
    vT = nc.dram_tensor("vT", [128, DT, S], F16, **kw)
    maskT = nc.dram_tensor("maskT", [128, KT_TILES, SQ], F16, **kw)
    w = {x: nc.dram_tensor(f"w{x}", [128, DT, DL], F16, **kw) for x in "qkv"}
    w["o"] = nc.dram_tensor("wo", [128, DTL, D], F16, **kw)
    b = {x: nc.dram_tensor(f"b{x}", [128, DTL], F32, **kw) for x in "qk"}
    b["v_rep"] = nc.dram_tensor("bv_rep", [128, HL, DK], F32, **kw)
    if timing:
        nc.dram_tensor("tinput", [1, 8], F32, kind="ExternalInput")
    outT = nc.dram_tensor("outT", [D, SQ], F32, kind="ExternalOutput")

    with tile.TileContext(nc) as tc:
        _build_tile(tc, qT, kT, vT, maskT, w, b, outT)
    nc.compile()
    return nc


def _build_tile(tc, qT, kT, vT, maskT, w, b, outT):
    nc = tc.nc

    with (
        tc.tile_pool(name="persist", bufs=1) as persist,
        tc.tile_pool(name="stage", bufs=2) as stage,
        tc.tile_pool(name="probs", bufs=2) as probs_pool,
        tc.tile_pool(name="small", bufs=2) as small,
    ):
        # ---- DMAs for the projection phase first ----
        wq_sb = persist.tile([128, DT, DL], F16)
        nc.sync.dma_start(out=wq_sb[:], in_=w["q"][:])
        xq_sb = persist.tile([128, DT, SQ], F16)
        nc.sync.dma_start(out=xq_sb[:], in_=qT[:])
        wk_sb = persist.tile([128, DT, DL], F16)
        nc.sync.dma_start(out=wk_sb[:], in_=w["k"][:])
        wv_sb = persist.tile([128, DT, DL], F16)
        nc.sync.dma_start(out=wv_sb[:], in_=w["v"][:])
        wo_sb = persist.tile([128, DTL, D], F16)
        nc.sync.dma_start(out=wo_sb[:], in_=w["o"][:])

        bias_sb = {}
        for x in "qk":
            bias_sb[x] = persist.tile([128, DTL], F32, name=f"bias_{x}", tag=f"bias_{x}")
            nc.sync.dma_start(out=bias_sb[x][:], in_=b[x][:])
        bv_rep = persist.tile([128, HL, DK], F32)
        nc.sync.dma_start(out=bv_rep[:], in_=b["v_rep"][:])

        ones_col = persist.tile([128, HL, 1], F16)
        nc.vector.memset(ones_col[:], 1.0)

        KT_sb = persist.tile([128, DTL, S], F16)
        V_sb = persist.tile([128, KT_TILES, HL, DK + 1], F16)
        QT_sb = persist.tile([128, DTL, SQ], F16)
        ctx_sb = persist.tile([128, DTL, SQ], F16)

        maskT_sb = persist.tile([128, KT_TILES, SQ], F16)

        # preload the exp table set while PE does projections
        warm16 = small.tile([1, 16], F16, tag="warm")
        nc.vector.memset(warm16[:], 0.0)
        nc.scalar.activation(
            out=warm16[:], in_=warm16[:],
            func=mybir.ActivationFunctionType.Exp, bias=0.0, scale=1.0,
        )

        with tc.tile_pool(name="pprojk", bufs=4, space="PSUM") as pprojk:
            # ---- Q projection -> QT_sb [128, 3, 1024] ----
            for d in range(DTL):
                for half in range(2):
                    ps = pprojk.tile([128, 512], F32, tag="pj")
                    for ka in range(DT):
                        nc.tensor.matmul(
                            ps[:],
                            wq_sb[:, ka, d * 128 : (d + 1) * 128],
                            xq_sb[:, ka, half * 512 : (half + 1) * 512],
                            start=(ka == 0),
                            stop=(ka == DT - 1),
                        )
                    nc.vector.tensor_scalar_add(
                        out=QT_sb[:, d, half * 512 : (half + 1) * 512],
                        in0=ps[:],
                        scalar1=bias_sb["q"][:, d : d + 1],
                    )

            # ---- K projection -> KT_sb [128, 3, 4096] ----
            for nch in range(NCH):
                x_sb = stage.tile([128, DT, 512], F16, tag="xT")
                nc.sync.dma_start(out=x_sb[:], in_=kT[:, :, nch * 512 : (nch + 1) * 512])
                for d in range(DTL):
                    ps = pprojk.tile([128, 512], F32, tag="pj")
                    for ka in range(DT):
                        nc.tensor.matmul(
                            ps[:],
                            wk_sb[:, ka, d * 128 : (d + 1) * 128],
                            x_sb[:, ka, :],
                            start=(ka == 0),
                            stop=(ka == DT - 1),
                        )
                    nc.vector.tensor_scalar_add(
                        out=KT_sb[:, d, nch * 512 : (nch + 1) * 512],
                        in0=ps[:],
                        scalar1=bias_sb["k"][:, d : d + 1],
                    )

        # ---- V projection chunk emitter (interleaved with attention) ----
        def emit_v_chunk(pool, nch):
            x_sb = stage.tile([128, DT, 512], F16, tag="xT")
            nc.sync.dma_start(out=x_sb[:], in_=vT[:, :, nch * 512 : (nch + 1) * 512])
            for rt in range(4):
                kt = nch * 4 + rt
                ps = pool.tile([128, DL], F32, name="psv", tag="pv")
                for ka in range(DT):
                    nc.tensor.matmul(
                        ps[:],
                        x_sb[:, ka, rt * 128 : (rt + 1) * 128],
                        wv_sb[:, ka, :],
                        start=(ka == 0),
                        stop=(ka == DT - 1),
                    )
                nc.vector.tensor_add(
                    out=V_sb[:, kt, :, 0:DK],
                    in0=ps[:].rearrange("p (h e) -> p h e", e=DK),
                    in1=bv_rep[:],
                )
                nc.vector.tensor_copy(out=V_sb[:, kt, :, DK : DK + 1], in_=ones_col[:])

        def emit_oproj(pool, half):
            for d in range(DT):
                ps = pool.tile([128, 512], F32, name="pso", tag="scB")
                for ka in range(DTL):
                    nc.tensor.matmul(
                        ps[:],
                        wo_sb[:, ka, d * 128 : (d + 1) * 128],
                        ctx_sb[:, ka, half * 512 : (half + 1) * 512],
                        start=(ka == 0),
                        stop=(ka == DTL - 1),
                    )
                o_sb = small.tile([128, 512], F32, tag="osb")
                nc.vector.tensor_copy(out=o_sb[:], in_=ps[:])
                nc.sync.dma_start(
                    out=outT[d * 128 : (d + 1) * 128, half * 512 : (half + 1) * 512],
                    in_=o_sb[:],
                )

        # ---- V projection (own pool, closes before attention) ----
        # mask halves DMA'd behind the V input stream: needed only at attention
        with tc.tile_pool(name="pprojv", bufs=2, space="PSUM") as pprojv:
            for nch in range(NCH):
                emit_v_chunk(pprojv, nch)
            nc.sync.dma_start(out=maskT_sb[:, :, 0:512], in_=maskT[:, :, 0:512])
            nc.sync.dma_start(out=maskT_sb[:, :, 512:SQ], in_=maskT[:, :, 512:SQ])

        # ---- attention (one head at a time) + early O proj per q half ----
        with tc.tile_pool(name="pattn", bufs=1, space="PSUM") as pattn:
            for qh in range(2):
                qs = qh * 512
                for h in range(HL):
                    p, po = h // 2, 64 * (h % 2)
                    ctx = pattn.tile([128, 512], F32, tag="ctx")
                    for kts in GROUPS:
                        nk = len(kts)
                        tag = "scA" if nk == 4 else "scB"
                        sc = pattn.tile([128, nk, 512], F32, name="sc", tag=tag)
                        pr = probs_pool.tile(
                            [128, nk, 512], F16, name="pr", tag="pr" + tag
                        )
                        for j, kt in enumerate(kts):
                            nc.tensor.matmul(
                                sc[:, j, :],
                                KT_sb[po : po + 64, p, kt * 128 : (kt + 1) * 128],
                                QT_sb[po : po + 64, p, qs : qs + 512],
                                start=True,
                                stop=True,
                            )
                        # exp straight from PSUM; 1/sqrt(dk) in the scale
                        nc.scalar.activation(
                            out=pr[:],
                            in_=sc[:],
                            func=mybir.ActivationFunctionType.Exp,
                            bias=0.0,
                            scale=float(1.0 / np.sqrt(DK)),
                        )
                        # {0,1} mask, one fp16 multiply per group
                        nc.vector.tensor_mul(
                            out=pr[:],
                            in0=pr[:],
                            in1=maskT_sb[:, kts[0] : kts[0] + nk, qs : qs + 512],
                        )
                        for j, kt in enumerate(kts):
                            nc.tensor.matmul(
                                ctx[0 : DK + 1, :],
                                V_sb[:, kt, h, :],
                                pr[:, j, :],
                                start=(kt == 0),
                                stop=(kt == KT_TILES - 1),
                                skip_group_check=True,
                            )
                    # normalize: rows 0..63 ctx^T, row 64 denominator
                    recip = small.tile([1, 512], F32, tag="recip")
                    nc.vector.reciprocal(out=recip[:], in_=ctx[DK : DK + 1, :])
                    recip_rep = small.tile([DK, 512], F32, tag="recip_rep")
                    nc.gpsimd.partition_broadcast(recip_rep[:], recip[:])
                    nc.vector.tensor_mul(
                        out=ctx_sb[po : po + 64, p, qs : qs + 512],
                        in0=ctx[0:DK, :],
                        in1=recip_rep[:],
                    )
                # O projection for this query half overlaps the next half;
                # its PSUM tile borrows the scB slot (3 banks >= 1)
                emit_oproj(pattn, qh)


def _tile_dm(x):
    """[D, N] -> [128, D//128, N] fp16 (partition-tiled over the first dim)."""
    d, n = x.shape
    return np.ascontiguousarray(
        x.reshape(d // 128, 128, n).swapaxes(0, 1).astype(np.float16)
    )


def _prep_inputs(q, k, v, mask, wq, bq, wk, bk, wv, bv, wo, bo):
    q = np.asarray(q, dtype=np.float32).reshape(S, D)
    k = np.asarray(k, dtype=np.float32).reshape(S, D)
    v = np.asarray(v, dtype=np.float32).reshape(S, D)
    mask = np.asarray(mask).reshape(S, S)
    wq, wk, wv, wo = (np.asarray(x, np.float32) for x in (wq, wk, wv, wo))
    bq, bk, bv = (np.asarray(x, np.float32) for x in (bq, bk, bv))

    kT_t = _tile_dm(k.T)  # [128, 6, 4096]
    vT_t = _tile_dm(v.T)

    qT_qb, maskT_qb = [], []
    for qb in range(NQB):
        qs, qe = qb * SQ, (qb + 1) * SQ
        qT_qb.append(_tile_dm(q[qs:qe, :].T))
        maskT_qb.append(
            np.ascontiguousarray(
                mask[qs:qe, :].T.reshape(KT_TILES, 128, SQ).swapaxes(0, 1)
            ).astype(np.float16)
        )

    w_hh, b_hh = [], []
    for hh in range(NHH):
        cs, ce = hh * DL, (hh + 1) * DL
        w_hh.append(
            {
                "wq": _tile_dm(wq[:, cs:ce]),
                "wk": _tile_dm(wk[:, cs:ce]),
                "wv": _tile_dm(wv[:, cs:ce]),
                "wo": np.ascontiguousarray(
                    wo[cs:ce, :].reshape(DTL, 128, D).swapaxes(0, 1)
                ).astype(np.float16),
            }
        )
        b_hh.append(
            {
                "bq": np.ascontiguousarray(bq[cs:ce].reshape(DTL, 128).T),
                "bk": np.ascontiguousarray(bk[cs:ce].reshape(DTL, 128).T),
                "bv_rep": np.ascontiguousarray(
                    np.broadcast_to(bv[cs:ce].reshape(1, HL, DK), (128, HL, DK))
                ),
            }
        )

    in_maps = []
    for c in range(NCORES):
        qb, hh = c // NHH, c % NHH
        m = {
            "qT": qT_qb[qb],
            "kT": kT_t,
            "vT": vT_t,
            "maskT": maskT_qb[qb],
            "wq": w_hh[hh]["wq"],
            "wk": w_hh[hh]["wk"],
            "wv": w_hh[hh]["wv"],
            "wo": w_hh[hh]["wo"],
            "bq": b_hh[hh]["bq"],
            "bk": b_hh[hh]["bk"],
            "bv_rep": b_hh[hh]["bv_rep"],
        }
        in_maps.append(m)
    return in_maps


def kernel(**inputs) -> np.ndarray:
    if "nc" not in _CACHE:
        _CACHE["nc"] = build_kernel()
    nc = _CACHE["nc"]
    in_maps = _prep_inputs(**inputs)
    res = bass_utils.run_bass_kernel_spmd(nc, in_maps, core_ids=list(range(NCORES)))
    bo = np.asarray(inputs["bo"], np.float32)
    blocks = []
    for qb in range(NQB):
        acc = res.results[qb * NHH]["outT"].astype(np.float32) + res.results[
            qb * NHH + 1
        ]["outT"].astype(np.float32)
        blocks.append(acc.T + bo)
    out = np.concatenate(blocks, axis=0)
    return out.reshape(B, S, D)
